# revision 29
# baseline (speedup 1.0000x reference)
"""Trainium2 Bass kernel: Luong-style attention with predictive alignment.

Math (see reference):
    h_t    = x[:, -1, :]                                   [B, H]
    t      = tanh(h_t @ W_p);  aligned = S*sigmoid(t @ v_p)
    scores[b,s] = sum_h x[b,s,h] * u[b,h],  u[b] = W_a @ h_t[b]
        (algebraic rewrite of (x @ W_a) . h_t -- avoids the B*S*H*H einsum)
    attn   = softmax(scores) * exp(-(pos-aligned)^2 / sigma2)
    ctx[b] = sum_s attn[b,s] * x[b,s,:]
    out    = tanh(concat(ctx, h_t) @ W_v)

Sharding: data-parallel over batch. 8 cores x 4 batches each; weights
replicated per core.

Per-core dataflow:
 - x shard streamed as 1 MiB chunks [128p, 2, 1024] (s = chunk*256 + p*2 + a)
 - scores via fused DVE scalar_tensor_tensor against u broadcast (exact fp32,
   reading the f32r-typed x tiles through a bitcast view)
 - softmax max/sum via PE transpose + ones-matmul partition reductions
 - gauss window folded into the exp: attn = exp(scores - m - ((pos-al)/sg)^2)
 - context/final/t matmuls in float32r (1 cyc/row vs 4 for fp32); u kept fp32
   because score precision feeds exp()
 - DMA order tuned so u (scores dependency) lands first: W_aT -> x b0 -> W_p
   -> x b1 -> x b2 -> W_v -> x b3 on the sync ring; small dependent DMAs ride
   the scalar ring to avoid head-of-line blocking.
"""

import math
from contextlib import ExitStack

import numpy as np

import concourse.bass as bass
import concourse.bass_isa as bass_isa
import concourse.mybir as mybir
import concourse.tile as tile
from concourse import bacc
from concourse.bass_utils import run_bass_kernel_spmd

B, S, H, SIZE = 32, 2048, 1024, 1024
NCORES = 8
BPC = B // NCORES          # batches per core
NCH = 8                    # x chunks per batch
SCH = S // NCH             # 256 sequence positions per chunk
A = 2                      # sub-slices (128 s-positions each) per chunk
COLS = NCH * A             # 16 score columns per batch
F32 = mybir.dt.float32
F32R = mybir.dt.float32r
SIGMA_SQ = 2.0 * (S / 2.0 / 2.0) ** 2    # D = S//2; 2*(D/2)^2 = 524288
INV_SG = 1.0 / math.sqrt(SIGMA_SQ)

_CACHE = {}
TRACE = False


def _build():
    AF = mybir.ActivationFunctionType
    OP = mybir.AluOpType
    nc = bacc.Bacc()

    x_s = nc.dram_tensor("x_s", [BPC, S, H], F32, kind="ExternalInput")
    w_p = nc.dram_tensor("w_p", [H, H], F32, kind="ExternalInput")
    w_at = nc.dram_tensor("w_at", [H, H], F32, kind="ExternalInput")
    w_v = nc.dram_tensor("w_v", [2 * H, SIZE], F32, kind="ExternalInput")
    htk = nc.dram_tensor("htk", [128, 8 * BPC], F32, kind="ExternalInput")
    vrep = nc.dram_tensor("vrep", [BPC, H], F32, kind="ExternalInput")
    posd = nc.dram_tensor("pos", [128, COLS], F32, kind="ExternalInput")
    idd = nc.dram_tensor("ident", [128, 128], F32, kind="ExternalInput")
    outd = nc.dram_tensor("out", [BPC, SIZE], F32, kind="ExternalOutput")

    with tile.TileContext(nc) as tc, ExitStack() as ctx:
        const = ctx.enter_context(tc.tile_pool(name="const", bufs=1))
        wts = ctx.enter_context(tc.tile_pool(name="wts", bufs=1))
        xs = ctx.enter_context(tc.tile_pool(name="xs", bufs=11))
        scr = ctx.enter_context(tc.tile_pool(name="scr", bufs=2))
        prodp = ctx.enter_context(tc.tile_pool(name="prodp", bufs=1))
        small = ctx.enter_context(tc.tile_pool(name="small", bufs=2))
        psA = ctx.enter_context(
            tc.tile_pool(name="psA", bufs=2, space=bass.MemorySpace.PSUM)
        )
        psCtx = ctx.enter_context(
            tc.tile_pool(name="psCtx", bufs=1, space=bass.MemorySpace.PSUM)
        )
        psT = ctx.enter_context(
            tc.tile_pool(name="psT", bufs=2, space=bass.MemorySpace.PSUM)
        )
        dpool = ctx.enter_context(
            tc.tile_pool(name="dram", bufs=1, space=bass.MemorySpace.DRAM)
        )

        # ---- constants / small inputs (sync ring: smalls first) ----
        combT = const.tile([128, 8 * BPC * 2], F32R)  # combined^T: [p, 4k+b]
        htk_f32 = const.tile([128, 8 * BPC], F32)     # fp32 copy for u lhsT
        v_sb = const.tile([BPC, H], F32)
        pos_sb = const.tile([128, COLS], F32)
        id_sb = const.tile([128, 128], F32)
        tta = const.tile([BPC, H], F32)
        alb = const.tile([BPC, 1], F32)
        out_sb = const.tile([BPC, SIZE], F32)

        nc.sync.dma_start(out=combT[:, 32:64], in_=htk[:, :].bitcast(F32R))
        nc.sync.dma_start(out=htk_f32, in_=htk[:, :])
        nc.sync.dma_start(out=v_sb, in_=vrep[:, :])
        nc.sync.dma_start(out=pos_sb, in_=posd[:, :])
        nc.sync.dma_start(out=id_sb, in_=idd[:, :])

        # ---- W_aT first: u is the critical dependency for scores ----
        wa_sb = wts.tile([128, 8, H], F32R, tag="w1")
        nc.sync.dma_start(
            out=wa_sb,
            in_=w_at[:, :].rearrange("(k p) j -> p k j", p=128).bitcast(F32R),
        )

        # u[b] broadcast across partitions, computed directly on PE: lhsT is
        # the h_t column replicated along its free dim (step-0 AP), so
        # out[p, h] = sum_k h_t[b,k] W_aT[k,h] = u[b,h] for every partition p.
        ubc_tiles = [None] * BPC

        def emit_ubc(b):
            ub_ps = psA.tile([128, H], F32, tag="pa", name=f"ubps_{b}")
            for k in range(8):
                c0 = combT[:, 32 + 4 * k + b : 32 + 4 * k + b + 1]
                lhs = bass.AP(
                    tensor=c0.tensor, offset=c0.offset, ap=[c0.ap[0], [0, 128]]
                )
                for h2 in range(2):
                    nc.tensor.matmul(
                        ub_ps[:, 512 * h2 : 512 * (h2 + 1)],
                        lhs,
                        wa_sb[:, k, 512 * h2 : 512 * (h2 + 1)],
                        start=(k == 0),
                        stop=(k == 7),
                    )
            ubc = scr.tile([128, H], F32, tag="ubc", name=f"ubc_{b}")
            nc.scalar.copy(ubc, ub_ps)
            ubc_tiles[b] = ubc

        emit_ubc(0)
        emit_ubc(1)

        # ---- x DMAs (sync ring) ----
        all_x = [[None] * NCH for _ in range(BPC)]

        def emit_x_dmas(b, cs):
            for c in cs:
                xt = xs.tile([128, A, H], F32R, tag="xt", name=f"xt_{b}_{c}")
                nc.sync.dma_start(
                    out=xt,
                    in_=x_s[b, c * SCH : (c + 1) * SCH, :]
                    .rearrange("(p a) h -> p a h", p=128)
                    .bitcast(F32R),
                )
                all_x[b][c] = xt

        emit_x_dmas(0, range(4))

        # ---- W_p + t/aligned (f32r matmul; lhsT = combT h_t half) ----
        wp_sb = wts.tile([128, 8, H], F32R, tag="w0")
        nc.sync.dma_start(
            out=wp_sb, in_=w_p[:, :].rearrange("(k p) j -> p k j", p=128).bitcast(F32R)
        )
        emit_x_dmas(0, range(4, NCH))
        ab_d = dpool.tile([BPC, 1], F32)

        def emit_aligned_section():
            # t = tanh(h_t @ W_p); aligned = S*sigmoid(t @ v_p). Emitted after
            # batch-0's score STTs so the W_p-dependent DVE op doesn't
            # head-of-line block the score stream.
            ps_t = psA.tile([BPC, H], F32, tag="pa")
            for k in range(8):
                lhs = combT[:, 32 + 4 * k : 32 + 4 * k + 4]
                for h2 in range(2):
                    nc.tensor.matmul(
                        ps_t[:, 512 * h2 : 512 * (h2 + 1)],
                        lhs,
                        wp_sb[:, k, 512 * h2 : 512 * (h2 + 1)],
                        start=(k == 0),
                        stop=(k == 7),
                    )
            nc.scalar.activation(out=tta, in_=ps_t, func=AF.Tanh)

            prod0 = prodp.tile([BPC, H], F32, tag="p0")
            al_r = small.tile([BPC, 1], F32, tag="alr")
            nc.vector.scalar_tensor_tensor(
                out=prod0,
                in0=tta,
                scalar=1.0,
                in1=v_sb,
                op0=OP.mult,
                op1=OP.mult,
                accum_out=al_r,
            )
            nc.scalar.activation(out=alb, in_=al_r, func=AF.Sigmoid)
            nc.scalar.mul(alb, alb, -float(S) * INV_SG)  # alb = -aligned/sg
            nc.scalar.dma_start(out=ab_d[:, :], in_=alb)

        # ---- per-batch: scores, softmax, context ----
        def batch_section(b, after_scores=None):
            ubc = ubc_tiles[b]
            sc_b = small.tile([128, COLS], F32, tag="scb", name=f"scb_{b}")
            for c in range(NCH):
                xt = all_x[b][c]
                for a in range(A):
                    prod = prodp.tile([128, H], F32, tag="p0", name=f"pr_{b}_{c}_{a}")
                    col = c * A + a
                    nc.vector.scalar_tensor_tensor(
                        out=prod,
                        in0=xt[:, a, :].bitcast(F32),
                        scalar=1.0,
                        in1=ubc,
                        op0=OP.mult,
                        op1=OP.mult,
                        accum_out=sc_b[:, col : col + 1],
                    )
            if after_scores is not None:
                after_scores()

            # softmax pieces: m (global max), Z = sum exp(s-m)
            mx_p = small.tile([128, 1], F32, tag="mxp", name=f"mxp_{b}")
            nc.vector.reduce_max(out=mx_p, in_=sc_b, axis=mybir.AxisListType.X)
            mcast = small.tile([128, 1], F32, tag="mcast", name=f"mcast_{b}")
            nc.gpsimd.partition_all_reduce(
                mcast, mx_p, channels=128, reduce_op=bass_isa.ReduceOp.max
            )
            negm = small.tile([128, 1], F32, tag="negm", name=f"negm_{b}")
            nc.scalar.mul(negm, mcast, -1.0)

            ew = small.tile([128, COLS], F32, tag="ew", name=f"ew_{b}")
            zp = small.tile([128, 1], F32, tag="zp", name=f"zp_{b}")
            nc.scalar.activation(
                out=ew, in_=sc_b, func=AF.Exp, bias=negm, scale=1.0, accum_out=zp
            )
            zsum = small.tile([128, 1], F32, tag="zsum", name=f"zsum_{b}")
            nc.gpsimd.partition_all_reduce(
                zsum, zp, channels=128, reduce_op=bass_isa.ReduceOp.add
            )
            zinv = small.tile([1, 1], F32, tag="zinv", name=f"zinv_{b}")
            nc.vector.reciprocal(zinv, zsum[0:1, :])

            # attn = exp(scores - m - ((pos - aligned)/sg)^2), Z folded later
            ab_b = small.tile([128, 1], F32, tag="abb", name=f"abb_{b}")
            nc.scalar.dma_start(
                out=ab_b, in_=ab_d[b : b + 1, :].to_broadcast((128, 1))
            )
            g2 = small.tile([128, COLS], F32, tag="g2", name=f"g2_{b}")
            nc.scalar.activation(
                out=g2, in_=pos_sb, func=AF.Square, bias=ab_b, scale=INV_SG
            )
            e_b = small.tile([128, COLS], F32, tag="eb", name=f"eb_{b}")
            nc.vector.tensor_sub(out=e_b, in0=sc_b, in1=g2)
            at_r = small.tile([128, COLS], F32R, tag="atr", name=f"atr_{b}")
            nc.scalar.activation(out=at_r, in_=e_b, func=AF.Exp, bias=negm, scale=1.0)

            # context[b] = (1/Z) * sum_s attn[s] x[s, :]   (f32r matmuls)
            ps_c = psCtx.tile([1, H], F32, tag="pc", name=f"pc_{b}")
            for c in range(NCH):
                for a in range(A):
                    col = c * A + a
                    for h2 in range(2):
                        nc.tensor.matmul(
                            ps_c[:, 512 * h2 : 512 * (h2 + 1)],
                            at_r[:, col : col + 1],
                            all_x[b][c][:, a, 512 * h2 : 512 * (h2 + 1)],
                            start=(col == 0),
                            stop=(col == COLS - 1),
                        )
            ctx_t = scr.tile([1, H], F32, tag="ctx", name=f"ctx_{b}")
            nc.scalar.activation(
                out=ctx_t, in_=ps_c, func=AF.Copy, bias=0.0, scale=zinv
            )
            # transpose context into combT columns [p, 4k+b] (rounds to f32r)
            for k in range(8):
                ps_ct = psT.tile([128, 1], F32, tag="pt", name=f"pct_{b}_{k}")
                nc.tensor.transpose(
                    ps_ct, ctx_t[0:1, 128 * k : 128 * (k + 1)], id_sb[0:1, 0:1]
                )
                nc.scalar.copy(combT[:, 4 * k + b : 4 * k + b + 1], ps_ct)

        batch_section(0, after_scores=emit_aligned_section)
        emit_ubc(2)
        emit_x_dmas(1, range(NCH))
        batch_section(1)
        emit_ubc(3)
        emit_x_dmas(2, range(NCH))
        batch_section(2)

        # W_v loads (reuse W_p/W_aT slots) before the last x batch on the
        # ring, so the final matmul's h_t half can run hidden under batch 3.
        wv0_sb = wts.tile([128, 8, SIZE], F32R, tag="w0")
        wv1_sb = wts.tile([128, 8, SIZE], F32R, tag="w1")
        nc.sync.dma_start(
            out=wv0_sb,
            in_=w_v[0:H, :].rearrange("(k p) o -> p k o", p=128).bitcast(F32R),
        )
        nc.sync.dma_start(
            out=wv1_sb,
            in_=w_v[H : 2 * H, :].rearrange("(k p) o -> p k o", p=128).bitcast(F32R),
        )
        emit_x_dmas(3, range(NCH))

        # ---- final: out = tanh(combined @ W_v)  (f32r) ----
        # h_t half (combT cols 32:64, known from the start) runs as soon as
        # W_v lands; the ctx half joins the same PSUM accumulation at the end.
        ps_o = psA.tile([BPC, SIZE], F32, tag="pa")
        for k in range(8, 16):
            lhs = combT[:, 4 * k : 4 * k + 4]
            for h2 in range(2):
                nc.tensor.matmul(
                    ps_o[:, 512 * h2 : 512 * (h2 + 1)],
                    lhs,
                    wv1_sb[:, k % 8, 512 * h2 : 512 * (h2 + 1)],
                    start=(k == 8),
                    stop=False,
                )

        batch_section(3)

        for k in range(8):
            lhs = combT[:, 4 * k : 4 * k + 4]
            for h2 in range(2):
                nc.tensor.matmul(
                    ps_o[:, 512 * h2 : 512 * (h2 + 1)],
                    lhs,
                    wv0_sb[:, k, 512 * h2 : 512 * (h2 + 1)],
                    start=False,
                    stop=(k == 7),
                )
        nc.scalar.activation(out=out_sb, in_=ps_o, func=AF.Tanh)
        nc.sync.dma_start(out=outd[:, :], in_=out_sb)

    nc.compile()
    return nc


def _host_prep(x, W_p, v_p, W_a, W_v):
    x = np.ascontiguousarray(np.asarray(x, dtype=np.float32))
    W_p = np.ascontiguousarray(np.asarray(W_p, dtype=np.float32))
    v_p = np.asarray(v_p, dtype=np.float32).reshape(-1)
    W_aT = np.ascontiguousarray(np.asarray(W_a, dtype=np.float32).T)
    W_v = np.ascontiguousarray(np.asarray(W_v, dtype=np.float32))

    h_all = np.ascontiguousarray(x[:, -1, :])  # [B, H]
    vrep = np.ascontiguousarray(np.broadcast_to(v_p.reshape(1, H), (BPC, H)))
    cols = np.arange(COLS)
    p = np.arange(128)
    pos = ((cols[None, :] // A) * SCH + p[:, None] * A + (cols[None, :] % A)).astype(
        np.float32
    )
    pos = np.ascontiguousarray(pos)
    ident = np.eye(128, dtype=np.float32)

    in_maps = []
    for c in range(NCORES):
        hT = h_all[BPC * c : BPC * (c + 1)].T  # [H, BPC]
        htk_a = np.ascontiguousarray(
            hT.reshape(8, 128, BPC).transpose(1, 0, 2).reshape(128, 8 * BPC)
        )
        in_maps.append(
            dict(
                x_s=np.ascontiguousarray(x[BPC * c : BPC * (c + 1)]),
                w_p=W_p,
                w_at=W_aT,
                w_v=W_v,
                htk=htk_a,
                vrep=vrep,
                pos=pos,
                ident=ident,
            )
        )
    return in_maps


def kernel(x, W_p, v_p, W_a, W_v):
    if "nc" not in _CACHE:
        _CACHE["nc"] = _build()
    nc = _CACHE["nc"]
    in_maps = _host_prep(x, W_p, v_p, W_a, W_v)
    res = run_bass_kernel_spmd(nc, in_maps, core_ids=list(range(NCORES)), trace=TRACE)
    _CACHE["last_results"] = res
    return np.concatenate([r["out"] for r in res.results], axis=0)


# revision 30
# speedup vs baseline: 1.0444x; 1.0444x over previous
"""Trainium2 Bass kernel: Luong-style attention with predictive alignment.

Math (see reference):
    h_t    = x[:, -1, :]                                   [B, H]
    t      = tanh(h_t @ W_p);  aligned = S*sigmoid(t @ v_p)
    scores[b,s] = sum_h x[b,s,h] * u[b,h],  u[b] = W_a @ h_t[b]
        (algebraic rewrite of (x @ W_a) . h_t -- avoids the B*S*H*H einsum)
    attn   = softmax(scores) * exp(-(pos-aligned)^2 / sigma2)
    ctx[b] = sum_s attn[b,s] * x[b,s,:]
    out    = tanh(concat(ctx, h_t) @ W_v)

Sharding: data-parallel over batch. 8 cores x 4 batches each; weights
replicated per core.

Per-core dataflow:
 - x shard streamed as 1 MiB chunks [128p, 2, 1024] (s = chunk*256 + p*2 + a)
 - scores via fused DVE scalar_tensor_tensor against u broadcast (exact fp32,
   reading the f32r-typed x tiles through a bitcast view)
 - softmax max/sum via PE transpose + ones-matmul partition reductions
 - gauss window folded into the exp: attn = exp(scores - m - ((pos-al)/sg)^2)
 - context/final/t matmuls in float32r (1 cyc/row vs 4 for fp32); u kept fp32
   because score precision feeds exp()
 - DMA order tuned so u (scores dependency) lands first: W_aT -> x b0 -> W_p
   -> x b1 -> x b2 -> W_v -> x b3 on the sync ring; small dependent DMAs ride
   the scalar ring to avoid head-of-line blocking.
"""

import math
from contextlib import ExitStack

import numpy as np

import concourse.bass as bass
import concourse.bass_isa as bass_isa
import concourse.mybir as mybir
import concourse.tile as tile
from concourse import bacc
from concourse.bass_utils import run_bass_kernel_spmd

B, S, H, SIZE = 32, 2048, 1024, 1024
NCORES = 8
BPC = B // NCORES          # batches per core
NCH = 8                    # x chunks per batch
SCH = S // NCH             # 256 sequence positions per chunk
A = 2                      # sub-slices (128 s-positions each) per chunk
COLS = NCH * A             # 16 score columns per batch
F32 = mybir.dt.float32
F32R = mybir.dt.float32r
SIGMA_SQ = 2.0 * (S / 2.0 / 2.0) ** 2    # D = S//2; 2*(D/2)^2 = 524288
INV_SG = 1.0 / math.sqrt(SIGMA_SQ)

_CACHE = {}
TRACE = False


def _build():
    AF = mybir.ActivationFunctionType
    OP = mybir.AluOpType
    nc = bacc.Bacc()

    x_s = nc.dram_tensor("x_s", [BPC, S, H], F32, kind="ExternalInput")
    w_p = nc.dram_tensor("w_p", [H, H], F32, kind="ExternalInput")
    w_at = nc.dram_tensor("w_at", [H, H], F32, kind="ExternalInput")
    w_v = nc.dram_tensor("w_v", [2 * H, SIZE], F32, kind="ExternalInput")
    htk = nc.dram_tensor("htk", [128, 8 * BPC], F32, kind="ExternalInput")
    vrep = nc.dram_tensor("vrep", [BPC, H], F32, kind="ExternalInput")
    posd = nc.dram_tensor("pos", [128, COLS], F32, kind="ExternalInput")
    idd = nc.dram_tensor("ident", [128, 128], F32, kind="ExternalInput")
    outd = nc.dram_tensor("out", [BPC, SIZE], F32, kind="ExternalOutput")

    with tile.TileContext(nc) as tc, ExitStack() as ctx:
        const = ctx.enter_context(tc.tile_pool(name="const", bufs=1))
        wts = ctx.enter_context(tc.tile_pool(name="wts", bufs=1))
        xs = ctx.enter_context(tc.tile_pool(name="xs", bufs=11))
        scr = ctx.enter_context(tc.tile_pool(name="scr", bufs=2))
        prodp = ctx.enter_context(tc.tile_pool(name="prodp", bufs=1))
        small = ctx.enter_context(tc.tile_pool(name="small", bufs=2))
        psA = ctx.enter_context(
            tc.tile_pool(name="psA", bufs=2, space=bass.MemorySpace.PSUM)
        )
        psCtx = ctx.enter_context(
            tc.tile_pool(name="psCtx", bufs=1, space=bass.MemorySpace.PSUM)
        )
        psT = ctx.enter_context(
            tc.tile_pool(name="psT", bufs=2, space=bass.MemorySpace.PSUM)
        )
        dpool = ctx.enter_context(
            tc.tile_pool(name="dram", bufs=1, space=bass.MemorySpace.DRAM)
        )

        # ---- constants / small inputs (sync ring: smalls first) ----
        combT = const.tile([128, 8 * BPC * 2], F32R)  # combined^T: [p, 4k+b]
        htk_f32 = const.tile([128, 8 * BPC], F32)     # fp32 copy for u lhsT
        v_sb = const.tile([BPC, H], F32)
        pos_sb = const.tile([128, COLS], F32)
        id_sb = const.tile([128, 128], F32)
        tta = const.tile([BPC, H], F32)
        alb = const.tile([BPC, 1], F32)
        out_sb = const.tile([BPC, SIZE], F32)

        nc.sync.dma_start(out=combT[:, 32:64], in_=htk[:, :].bitcast(F32R))
        nc.sync.dma_start(out=htk_f32, in_=htk[:, :])
        nc.sync.dma_start(out=v_sb, in_=vrep[:, :])
        nc.sync.dma_start(out=pos_sb, in_=posd[:, :])
        nc.sync.dma_start(out=id_sb, in_=idd[:, :])

        # ---- W_aT first: u is the critical dependency for scores ----
        wa_sb = wts.tile([128, 8, H], F32R, tag="w1")
        nc.sync.dma_start(
            out=wa_sb,
            in_=w_at[:, :].rearrange("(k p) j -> p k j", p=128).bitcast(F32R),
        )

        # u[b] broadcast across partitions, computed directly on PE: lhsT is
        # the h_t column replicated along its free dim (step-0 AP), so
        # out[p, h] = sum_k h_t[b,k] W_aT[k,h] = u[b,h] for every partition p.
        ubc_tiles = [None] * BPC

        def emit_ubc(b):
            ub_ps = psA.tile([128, H], F32, tag="pa", name=f"ubps_{b}")
            for k in range(8):
                c0 = combT[:, 32 + 4 * k + b : 32 + 4 * k + b + 1]
                lhs = bass.AP(
                    tensor=c0.tensor, offset=c0.offset, ap=[c0.ap[0], [0, 128]]
                )
                for h2 in range(2):
                    nc.tensor.matmul(
                        ub_ps[:, 512 * h2 : 512 * (h2 + 1)],
                        lhs,
                        wa_sb[:, k, 512 * h2 : 512 * (h2 + 1)],
                        start=(k == 0),
                        stop=(k == 7),
                    )
            ubc = scr.tile([128, H], F32, tag="ubc", name=f"ubc_{b}")
            nc.scalar.copy(ubc, ub_ps)
            ubc_tiles[b] = ubc

        emit_ubc(0)
        emit_ubc(1)

        # ---- x DMAs (sync ring) ----
        all_x = [[None] * NCH for _ in range(BPC)]

        def emit_x_dmas(b, cs):
            for c in cs:
                xt = xs.tile([128, A, H], F32R, tag="xt", name=f"xt_{b}_{c}")
                nc.sync.dma_start(
                    out=xt,
                    in_=x_s[b, c * SCH : (c + 1) * SCH, :]
                    .rearrange("(p a) h -> p a h", p=128)
                    .bitcast(F32R),
                )
                all_x[b][c] = xt

        emit_x_dmas(0, range(4))

        # ---- W_p + t/aligned (f32r matmul; lhsT = combT h_t half) ----
        wp_sb = wts.tile([128, 8, H], F32R, tag="w0")
        nc.sync.dma_start(
            out=wp_sb, in_=w_p[:, :].rearrange("(k p) j -> p k j", p=128).bitcast(F32R)
        )
        emit_x_dmas(0, range(4, NCH))
        ab_d = dpool.tile([BPC, 1], F32)

        def emit_aligned_section():
            # t = tanh(h_t @ W_p); aligned = S*sigmoid(t @ v_p). Emitted after
            # batch-0's score STTs so the W_p-dependent DVE op doesn't
            # head-of-line block the score stream.
            ps_t = psA.tile([BPC, H], F32, tag="pa")
            for k in range(8):
                lhs = combT[:, 32 + 4 * k : 32 + 4 * k + 4]
                for h2 in range(2):
                    nc.tensor.matmul(
                        ps_t[:, 512 * h2 : 512 * (h2 + 1)],
                        lhs,
                        wp_sb[:, k, 512 * h2 : 512 * (h2 + 1)],
                        start=(k == 0),
                        stop=(k == 7),
                    )
            nc.scalar.activation(out=tta, in_=ps_t, func=AF.Tanh)

            prod0 = prodp.tile([BPC, H], F32, tag="p0")
            al_r = small.tile([BPC, 1], F32, tag="alr")
            nc.vector.scalar_tensor_tensor(
                out=prod0,
                in0=tta,
                scalar=1.0,
                in1=v_sb,
                op0=OP.mult,
                op1=OP.mult,
                accum_out=al_r,
            )
            nc.scalar.activation(out=alb, in_=al_r, func=AF.Sigmoid)
            nc.scalar.mul(alb, alb, -float(S) * INV_SG)  # alb = -aligned/sg
            nc.scalar.dma_start(out=ab_d[:, :], in_=alb)

        # ---- per-batch: scores, softmax, context ----
        def batch_section(b, after_scores=None):
            ubc = ubc_tiles[b]
            sc_b = small.tile([128, COLS], F32, tag="scb", name=f"scb_{b}")
            for c in range(NCH):
                xt = all_x[b][c]
                for a in range(A):
                    prod = prodp.tile([128, H], F32, tag="p0", name=f"pr_{b}_{c}_{a}")
                    col = c * A + a
                    nc.vector.scalar_tensor_tensor(
                        out=prod,
                        in0=xt[:, a, :].bitcast(F32),
                        scalar=1.0,
                        in1=ubc,
                        op0=OP.mult,
                        op1=OP.mult,
                        accum_out=sc_b[:, col : col + 1],
                    )
            if after_scores is not None:
                after_scores()

            # softmax pieces: m (global max), Z = sum exp(s-m)
            mx_p = small.tile([128, 1], F32, tag="mxp", name=f"mxp_{b}")
            nc.vector.reduce_max(out=mx_p, in_=sc_b, axis=mybir.AxisListType.X)
            mcast = small.tile([128, 1], F32, tag="mcast", name=f"mcast_{b}")
            nc.gpsimd.partition_all_reduce(
                mcast, mx_p, channels=128, reduce_op=bass_isa.ReduceOp.max
            )
            negm = small.tile([128, 1], F32, tag="negm", name=f"negm_{b}")
            nc.scalar.mul(negm, mcast, -1.0)

            ew = small.tile([128, COLS], F32, tag="ew", name=f"ew_{b}")
            zp = small.tile([128, 1], F32, tag="zp", name=f"zp_{b}")
            nc.scalar.activation(
                out=ew, in_=sc_b, func=AF.Exp, bias=negm, scale=1.0, accum_out=zp
            )
            zsum = small.tile([128, 1], F32, tag="zsum", name=f"zsum_{b}")
            nc.gpsimd.partition_all_reduce(
                zsum, zp, channels=128, reduce_op=bass_isa.ReduceOp.add
            )
            zinv = small.tile([1, 1], F32, tag="zinv", name=f"zinv_{b}")
            nc.vector.reciprocal(zinv, zsum[0:1, :])

            # attn = exp(scores - m - ((pos - aligned)/sg)^2), Z folded later
            ab_b = small.tile([128, 1], F32, tag="abb", name=f"abb_{b}")
            nc.scalar.dma_start(
                out=ab_b, in_=ab_d[b : b + 1, :].to_broadcast((128, 1))
            )
            g2 = small.tile([128, COLS], F32, tag="g2", name=f"g2_{b}")
            nc.scalar.activation(
                out=g2, in_=pos_sb, func=AF.Square, bias=ab_b, scale=INV_SG
            )
            e_b = small.tile([128, COLS], F32, tag="eb", name=f"eb_{b}")
            nc.vector.tensor_sub(out=e_b, in0=sc_b, in1=g2)
            at_r = small.tile([128, COLS], F32R, tag="atr", name=f"atr_{b}")
            nc.scalar.activation(out=at_r, in_=e_b, func=AF.Exp, bias=negm, scale=1.0)

            # context[b] = (1/Z) * sum_s attn[s] x[s, :]   (f32r matmuls)
            ps_c = psCtx.tile([1, H], F32, tag="pc", name=f"pc_{b}")
            for c in range(NCH):
                for a in range(A):
                    col = c * A + a
                    for h2 in range(2):
                        nc.tensor.matmul(
                            ps_c[:, 512 * h2 : 512 * (h2 + 1)],
                            at_r[:, col : col + 1],
                            all_x[b][c][:, a, 512 * h2 : 512 * (h2 + 1)],
                            start=(col == 0),
                            stop=(col == COLS - 1),
                        )
            ctx_t = scr.tile([1, H], F32, tag="ctx", name=f"ctx_{b}")
            nc.scalar.activation(
                out=ctx_t, in_=ps_c, func=AF.Copy, bias=0.0, scale=zinv
            )
            # transpose context into combT columns [p, 4k+b] (rounds to f32r)
            for k in range(8):
                ps_ct = psT.tile([128, 1], F32, tag="pt", name=f"pct_{b}_{k}")
                nc.tensor.transpose(
                    ps_ct, ctx_t[0:1, 128 * k : 128 * (k + 1)], id_sb[0:1, 0:1]
                )
                nc.scalar.copy(combT[:, 4 * k + b : 4 * k + b + 1], ps_ct)

        batch_section(0, after_scores=emit_aligned_section)
        emit_ubc(2)
        emit_x_dmas(1, range(NCH))
        batch_section(1)
        emit_ubc(3)
        emit_x_dmas(2, range(NCH))
        batch_section(2)
        emit_x_dmas(3, range(NCH))

        # W_v loads (reuse W_p/W_aT slots), after the last x batch on the ring
        wv0_sb = wts.tile([128, 8, SIZE], F32R, tag="w0")
        wv1_sb = wts.tile([128, 8, SIZE], F32R, tag="w1")
        nc.sync.dma_start(
            out=wv0_sb,
            in_=w_v[0:H, :].rearrange("(k p) o -> p k o", p=128).bitcast(F32R),
        )
        nc.sync.dma_start(
            out=wv1_sb,
            in_=w_v[H : 2 * H, :].rearrange("(k p) o -> p k o", p=128).bitcast(F32R),
        )

        # ---- final: out = tanh(combined @ W_v)  (f32r) ----
        # h_t half (combT cols 32:64, known from the start) runs as soon as
        # W_v lands; the ctx half joins the same PSUM accumulation at the end.
        ps_o = psA.tile([BPC, SIZE], F32, tag="pa")
        for k in range(8, 16):
            lhs = combT[:, 4 * k : 4 * k + 4]
            for h2 in range(2):
                nc.tensor.matmul(
                    ps_o[:, 512 * h2 : 512 * (h2 + 1)],
                    lhs,
                    wv1_sb[:, k % 8, 512 * h2 : 512 * (h2 + 1)],
                    start=(k == 8),
                    stop=False,
                )

        batch_section(3)

        for k in range(8):
            lhs = combT[:, 4 * k : 4 * k + 4]
            for h2 in range(2):
                nc.tensor.matmul(
                    ps_o[:, 512 * h2 : 512 * (h2 + 1)],
                    lhs,
                    wv0_sb[:, k, 512 * h2 : 512 * (h2 + 1)],
                    start=False,
                    stop=(k == 7),
                )
        nc.scalar.activation(out=out_sb, in_=ps_o, func=AF.Tanh)
        nc.sync.dma_start(out=outd[:, :], in_=out_sb)

    nc.compile()
    return nc


def _host_prep(x, W_p, v_p, W_a, W_v):
    x = np.ascontiguousarray(np.asarray(x, dtype=np.float32))
    W_p = np.ascontiguousarray(np.asarray(W_p, dtype=np.float32))
    v_p = np.asarray(v_p, dtype=np.float32).reshape(-1)
    W_aT = np.ascontiguousarray(np.asarray(W_a, dtype=np.float32).T)
    W_v = np.ascontiguousarray(np.asarray(W_v, dtype=np.float32))

    h_all = np.ascontiguousarray(x[:, -1, :])  # [B, H]
    vrep = np.ascontiguousarray(np.broadcast_to(v_p.reshape(1, H), (BPC, H)))
    cols = np.arange(COLS)
    p = np.arange(128)
    pos = ((cols[None, :] // A) * SCH + p[:, None] * A + (cols[None, :] % A)).astype(
        np.float32
    )
    pos = np.ascontiguousarray(pos)
    ident = np.eye(128, dtype=np.float32)

    in_maps = []
    for c in range(NCORES):
        hT = h_all[BPC * c : BPC * (c + 1)].T  # [H, BPC]
        htk_a = np.ascontiguousarray(
            hT.reshape(8, 128, BPC).transpose(1, 0, 2).reshape(128, 8 * BPC)
        )
        in_maps.append(
            dict(
                x_s=np.ascontiguousarray(x[BPC * c : BPC * (c + 1)]),
                w_p=W_p,
                w_at=W_aT,
                w_v=W_v,
                htk=htk_a,
                vrep=vrep,
                pos=pos,
                ident=ident,
            )
        )
    return in_maps


def kernel(x, W_p, v_p, W_a, W_v):
    if "nc" not in _CACHE:
        _CACHE["nc"] = _build()
    nc = _CACHE["nc"]
    in_maps = _host_prep(x, W_p, v_p, W_a, W_v)
    res = run_bass_kernel_spmd(nc, in_maps, core_ids=list(range(NCORES)), trace=TRACE)
    _CACHE["last_results"] = res
    return np.concatenate([r["out"] for r in res.results], axis=0)
